# revision 49
# baseline (speedup 1.0000x reference)
"""Sliding-window GQA causal self-attention (ALiBi) Trainium2 Bass kernel.

Problem: B=2, T=4096, C=1024, H=16, HKV=4 (GQA G=4), D=64, window W=512,
fused qkv projection + sliding-window attention + output projection.

Sharding: data-parallel over (batch x T/4) -> 8 cores. Each core computes
1024 query rows of one batch plus a 512-row k/v halo. No collectives.

Per-core dataflow (q/k in float32r; p/v/attnT/wo in bf16):
  - x arrives host-transposed; xT streamed in 512-column time slices
  - qT/kT computed transposed (stationary wqkv chunk, streaming xT);
    v computed natural and stored as 65-wide blocks (64 dims + ones col)
  - scores computed TRANSPOSED per 128-row k-chunk: sT[k, q] via
    lhsT=kT chunk, rhs=qT window; 3 augmentation rows fold in the ALiBi
    bias (rank-2 in absolute coords) and the left-edge -1e9 penalty
  - exp (no max-subtraction: scores are N(0,~6.5), exp cannot overflow
    at <13 sigma; softmax shift-invariance keeps this exact) writes the
    bf16 p slab directly in PV layout -- no PE transposes needed
  - window mask applied multiplicatively (0/1 triangles) on the bf16 slab
  - PV: per q-half, variable-width accumulation over 8 k-chunks covering
    only the valid band; ones-column of v gives the row-sum for free
  - normalize: reciprocal row -> PE broadcast [64,512] -> one DVE multiply
    that writes attnT (even heads) or a staging tile DMA'd to the upper
    64 partitions (odd heads)
  - out = attnT.T @ wo with wo streamed in bf16 chunks
"""

import math
from contextlib import ExitStack

import numpy as np
import ml_dtypes

import concourse.bass as bass
from concourse import bacc
import concourse.mybir as mybir
import concourse.tile as tile
from concourse.bass_utils import run_bass_kernel_spmd

F32 = mybir.dt.float32
F32R = mybir.dt.float32r
BF16 = mybir.dt.bfloat16

B, T, C = 2, 4096, 1024
H, HKV, G, D = 16, 4, 4, 64
W = 512
NCORES = 8
RT = 1024              # own query rows per core
KR = RT + W            # k/v slab rows (512 halo + 1024 own)
NKC = KR // 128        # 12 k-chunks of 128
SCALE = D ** -0.5      # 0.125, exact power of two
NEG = -1e9
KCOL0 = C              # wqkv col offset of k
VCOL0 = C + HKV * D    # wqkv col offset of v
VB = 65                # v block width (64 dims + ones column)
SLOT = 640             # p-slab slot width per k-chunk (wcol coords)

Exp = mybir.ActivationFunctionType.Exp

DEBUG = False


def alibi_slopes(n_head: int) -> np.ndarray:
    def slopes_power_of_2(n):
        start = 2.0 ** (-(2.0 ** (-(math.log2(n) - 3))))
        return [start * start ** i for i in range(n)]

    if float(math.log2(n_head)).is_integer():
        s = slopes_power_of_2(n_head)
    else:
        closest = 2 ** math.floor(math.log2(n_head))
        s = slopes_power_of_2(closest)
        s2 = slopes_power_of_2(2 * closest)
        s += s2[0::2][: n_head - closest]
    return np.array(s, dtype=np.float32)


def _ap(base, extra_off, dims):
    """AP with the partition dim of `base` and custom free dims."""
    return bass.AP(tensor=base.tensor, offset=base.offset + extra_off,
                   ap=[list(base.ap[0])] + dims)


def _dram_ap(t, off, dims):
    return bass.AP(tensor=t.tensor, offset=t.offset + off, ap=dims)


# per-chunk written window in wcol coords: [woff, woff+wlen)
def chunk_window(kc):
    wlo = max(0, 128 * kc - 512)          # q range [wlo, whi)
    whi = min(RT, 128 * kc + 128)
    woff = wlo - (128 * kc - 512)         # = max(0, 512 - 128*kc)
    return wlo, whi, woff, whi - wlo


def build_nc(loop: int = 1) -> bacc.Bacc:
    nc = bacc.Bacc("TRN2", target_bir_lowering=False)

    xs = nc.dram_tensor("xs", [C, KR], F32R, kind="ExternalInput")  # host-transposed
    wqkv = nc.dram_tensor("wqkv", [C, C + 2 * HKV * D], F32R, kind="ExternalInput")
    wo = nc.dram_tensor("wo", [C, C], BF16, kind="ExternalInput")
    qaug = nc.dram_tensor("qaug", [H, 3, RT], F32R, kind="ExternalInput")
    kaug = nc.dram_tensor("kaug", [3, HKV * KR], F32R, kind="ExternalInput")
    wnegT = nc.dram_tensor("wnegT", [128, 128], F32R, kind="ExternalInput")
    ident = nc.dram_tensor("ident", [128, 128], F32R, kind="ExternalInput")
    w1ab = nc.dram_tensor("w1ab", [128, 256], BF16, kind="ExternalInput")
    cns = nc.dram_tensor("cns", [3, 512], F32R, kind="ExternalInput")
    out = nc.dram_tensor("out", [RT, C], F32, kind="ExternalOutput")
    if DEBUG:
        dqT = nc.dram_tensor("dqT", [128, H * RT], F32, kind="ExternalOutput")
        dkT = nc.dram_tensor("dkT", [128, HKV * KR], F32, kind="ExternalOutput")
        dvsl = nc.dram_tensor("dvsl", [128, NKC * HKV * VB], BF16, kind="ExternalOutput")
        dslab = nc.dram_tensor("dslab", [128, NKC * SLOT], BF16, kind="ExternalOutput")
        dattnT = nc.dram_tensor("dattnT", [128, 8 * RT], BF16, kind="ExternalOutput")
        drecs = nc.dram_tensor("drecs", [2, 512], F32, kind="ExternalOutput")

    with tile.TileContext(nc) as tc:
      for _rep in range(loop):
        with ExitStack() as ctx:
            persist = ctx.enter_context(tc.tile_pool(name="persist", bufs=1))

            qT = persist.tile([128, H * RT], F32R)      # [0:64] data, [64:67] aug
            kT = persist.tile([128, HKV * KR], F32R)    # [0:64] data, [64:67] aug
            vsl = persist.tile([128, NKC * HKV * VB], BF16)  # v natural, kc-major
            attnT = persist.tile([128, 8 * RT], BF16)   # [c in chunk, cc*RT + t]
            wnegT_sb = persist.tile([128, 128], F32R)   # -1e9 left tri^T
            id_sb = persist.tile([128, 128], F32R)
            w1ab_sb = persist.tile([128, 256], BF16)    # [0/1 left | right] tri
            # broadcast weights: row 64 = 1, row 65 = 0 (contract-2 bcast mm).
            # Engine ops need partition base % 32 == 0: zero an aligned range
            # first, then overwrite row 64.
            bwt = persist.tile([128, 64], F32R)
            nc.gpsimd.dma_start(bwt[64:66, 0:64],
                              _dram_ap(cns[0:2, 0:64], 0, [[512, 2], [1, 64]]))
            # two recs tiles, row 65 zeroed once (killed by 0-weight in bcast)
            recsA = persist.tile([128, 512], F32R)
            recsB = persist.tile([128, 512], F32R)
            recsT = [recsA, recsB]
            nc.gpsimd.dma_start(recsA[65:66, :], cns[1:2, :])
            nc.gpsimd.dma_start(recsB[65:66, :], cns[2:3, :])

            # aux loads on the Pool (SWDGE) queue; SP queue stays clear for
            # the critical-path xs/wv loads
            nc.gpsimd.dma_start(wnegT_sb, wnegT[:, :])
            nc.gpsimd.dma_start(id_sb, ident[:, :])
            nc.gpsimd.dma_start(w1ab_sb, w1ab[:, :])
            # qaug: dst rows 64:67 of qT over all heads, one DMA
            # (src/dst dim structures must match: both (3, H, RT))
            nc.gpsimd.dma_start(
                _ap(qT[64:67, 0:1], 0, [[RT, H], [1, RT]]),
                _dram_ap(qaug[0, :, :], 0, [[RT, 3], [3 * RT, H], [1, RT]]))
            # kaug: pre-tiled [3, 4*KR] on host, one DMA
            nc.gpsimd.dma_start(kT[64:67, 0:HKV * KR], kaug[:, :])
            # ones columns of vsl (col 64 of each 65-wide block)
            for kcc in range(NKC):
                base = vsl[:, 0:1]
                nc.vector.memset(
                    _ap(base, kcc * HKV * VB + 64, [[VB, HKV], [1, 1]]), 1.0)

            # ---------------- Phase Q: qkv projection ----------------
            with tc.tile_pool(name="xTp", bufs=3) as xTp, \
                 tc.tile_pool(name="stg", bufs=2) as stg, \
                 tc.tile_pool(name="wqp", bufs=2) as wqp, \
                 tc.tile_pool(name="wvp", bufs=1) as wvp, \
                 tc.tile_pool(name="psQK", bufs=4, space="PSUM") as psQK, \
                 tc.tile_pool(name="psV", bufs=3, space="PSUM") as psV:

                wv = wvp.tile([128, 8 * 256], F32R)
                nc.sync.dma_start(
                    _ap(wv[:, 0:1], 0, [[256, 8], [1, 256]]),
                    _dram_ap(wqkv[:, :], VCOL0,
                             [[C + 512, 128], [128 * (C + 512), 8], [1, 256]]))

                xTts = {}

                def slice_dma(ts):
                    xTt = xTp.tile([128, 8 * 512], F32R, tag="xts")
                    # one DMA per 128-col tki chunk so compute starts early
                    for tki in range(4):
                        nc.sync.dma_start(
                            _ap(xTt[:, 0:1], tki * 128, [[512, 8], [1, 128]]),
                            _dram_ap(xs[:, :], ts * 512 + tki * 128,
                                     [[KR, 128], [128 * KR, 8], [1, 128]]))
                    xTts[ts] = xTt

                def slice_v(ts):
                    xTt = xTts[ts]
                    for tki in range(4):
                        tk = ts * 4 + tki
                        psv = psV.tile([128, 256], F32, tag="vps")
                        for cc in range(8):
                            nc.tensor.matmul(
                                psv,
                                lhsT=xTt[:, cc * 512 + tki * 128:cc * 512 + (tki + 1) * 128],
                                rhs=wv[:, cc * 256:(cc + 1) * 256],
                                start=(cc == 0), stop=(cc == 7))
                        # scatter 4 kv blocks into 65-wide slots (bf16)
                        nc.vector.tensor_copy(
                            _ap(vsl[:, 0:1], tk * HKV * VB, [[VB, HKV], [1, 64]]),
                            psv.rearrange("p (a b) -> p a b", b=64))

                def load_wq(fc2):
                    # fc2 0..3: q feature pairs; fc2 4: k features (both kv pairs)
                    fcol = fc2 * 256 if fc2 < 4 else KCOL0
                    wq = wqp.tile([128, 8 * 256], F32R, tag="wqf")
                    nc.sync.dma_start(
                        _ap(wq[:, 0:1], 0, [[256, 8], [1, 256]]),
                        _dram_ap(wqkv[:, :], fcol,
                                 [[C + 512, 128], [128 * (C + 512), 8], [1, 256]]))
                    return wq

                def qk_slab(fc2, ts_list, wq=None):
                    if wq is None:
                        wq = load_wq(fc2)
                    for ts in ts_list:
                        for fi in range(2):
                            fc = fc2 * 2 + fi
                            ps = psQK.tile([128, 512], F32, tag="qkps")
                            for cc in range(8):
                                nc.tensor.matmul(
                                    ps,
                                    lhsT=wq[:, cc * 256 + fi * 128:cc * 256 + (fi + 1) * 128],
                                    rhs=xTts[ts][:, cc * 512:(cc + 1) * 512],
                                    start=(cc == 0), stop=(cc == 7))
                            st = stg.tile([128, 512], F32R, tag="stg")
                            nc.scalar.copy(st[64:128, :], ps[64:128, :])
                            if fc2 < 4:
                                h0, h1 = 2 * fc, 2 * fc + 1
                                toff = (ts - 1) * 512
                                nc.vector.tensor_copy(
                                    qT[0:64, h0 * RT + toff:h0 * RT + toff + 512],
                                    ps[0:64, :])
                                nc.scalar.dma_start(
                                    qT[0:64, h1 * RT + toff:h1 * RT + toff + 512],
                                    st[64:128, :])
                            else:
                                kv0, kv1 = 2 * fi, 2 * fi + 1
                                toff = ts * 512
                                nc.vector.tensor_copy(
                                    kT[0:64, kv0 * KR + toff:kv0 * KR + toff + 512],
                                    ps[0:64, :])
                                nc.scalar.dma_start(
                                    kT[0:64, kv1 * KR + toff:kv1 * KR + toff + 512],
                                    st[64:128, :])

                slice_dma(0)
                wqk = load_wq(4)
                slice_dma(1)
                slice_v(0)
                qk_slab(4, [0], wqk)
                slice_v(1)
                slice_dma(2)
                qk_slab(4, [1], wqk)
                slice_v(2)
                qk_slab(4, [2], wqk)
                for fc2 in range(4):
                    qk_slab(fc2, [1, 2])

            # ---------------- Phase A: attention ----------------
            with tc.tile_pool(name="slabp", bufs=3) as slabp, \
                 tc.tile_pool(name="stO", bufs=2) as stO, \
                 tc.tile_pool(name="bcsp", bufs=2) as bcsp, \
                 tc.tile_pool(name="psS", bufs=2, space="PSUM") as psS, \
                 tc.tile_pool(name="psOB", bufs=3, space="PSUM") as psOB, \
                 tc.tile_pool(name="psB", bufs=1, space="PSUM") as psB:

                if DEBUG:
                    nc.sync.dma_start(dqT[:, :], qT[:, :].bitcast(F32))
                    nc.sync.dma_start(dkT[:, :], kT[:, :].bitcast(F32))
                    nc.sync.dma_start(dvsl[:, :], vsl[:, :])

                def score_chunk(h, kv, kc, slab):
                    wlo, whi, woff, wlen = chunk_window(kc)
                    stile = psS.tile([128, SLOT], F32, tag="sc")
                    kstat = kT[0:67, kv * KR + kc * 128:kv * KR + (kc + 1) * 128]
                    w0 = min(wlen, 512)
                    ltri = kc >= 4   # left tri: future keys
                    nc.tensor.matmul(
                        stile[:, 0:w0], lhsT=kstat,
                        rhs=qT[0:67, h * RT + wlo:h * RT + wlo + w0],
                        start=True, stop=not ltri,
                        skip_group_check=True)
                    if wlen > 512:
                        nc.tensor.matmul(
                            stile[:, 512:wlen], lhsT=kstat,
                            rhs=qT[0:67, h * RT + wlo + 512:h * RT + whi],
                            start=True, stop=True,
                            skip_group_check=True)
                    # Left tri = FUTURE keys (ALiBi bias positive, exp would
                    # overflow): -1e9 accumulated via PE (lhsT = tri^T,
                    # rhs = I) -> exp gives exact zeros.
                    if ltri:
                        nc.tensor.matmul(
                            stile[:, 0:128],
                            lhsT=wnegT_sb[:, 0:128], rhs=id_sb,
                            start=False, stop=True,
                            skip_group_check=True)
                    # No max-subtraction: valid scores are N(0, ~6.5);
                    # exp overflow needs ~13 sigma; bf16 range is ample.
                    nc.scalar.activation(
                        slab[:, kc * SLOT + woff:kc * SLOT + woff + wlen],
                        stile[:, 0:wlen], Exp, bias=0.0)
                    # Right tri = beyond-window past keys (finite): zero
                    # multiplicatively on the idle Pool engine.
                    if kc <= 7:
                        tri = slab[:, kc * SLOT + 512:kc * SLOT + 640]
                        nc.gpsimd.tensor_mul(tri, tri, w1ab_sb[:, 128:256])

                def pv(h, kv, half, slab):
                    oT = psOB.tile([VB, 512], F32, tag="ps_ob")
                    # s=3 first: full 512-wide, zeroes the bank
                    order = [3, 0, 1, 2, 4, 5, 6, 7]
                    for i, s in enumerate(order):
                        kc = 4 * half + s
                        c0 = max(0, 128 * s - 512)
                        c1 = min(512, 128 * s + 128)
                        rw0 = max(512 - 128 * s, 0)
                        nc.tensor.matmul(
                            oT[:, c0:c1],
                            lhsT=vsl[:, kc * HKV * VB + kv * VB:
                                     kc * HKV * VB + (kv + 1) * VB],
                            rhs=slab[:, kc * SLOT + rw0:kc * SLOT + rw0 + (c1 - c0)],
                            start=(i == 0), stop=(i == 7),
                            skip_group_check=True)
                    return oT

                _rec_i = [0]

                def norm(h, half, oT, stg_t):
                    cc = h // 2
                    recs = recsT[_rec_i[0] % 2]
                    _rec_i[0] += 1
                    with nc.allow_low_precision(reason="recip rounded to f32r"):
                        nc.vector.reciprocal(recs[64:65, :], oT[64:65, :])
                    bc = psB.tile([64, 512], F32, tag="bc")
                    nc.tensor.matmul(bc, lhsT=bwt[64:66, 0:64],
                                     rhs=recs[64:66, :],
                                     start=True, stop=True)
                    # TT can take at most one PSUM operand: stage bc
                    bcs = bcsp.tile([64, 512], F32, tag="bcs")
                    nc.vector.tensor_copy(bcs, bc)
                    if DEBUG and h == 0:
                        nc.sync.dma_start(
                            drecs[half:half + 1, :], recs[64:65, :].bitcast(F32))
                    cb = cc * RT + half * 512
                    if h % 2 == 0:
                        nc.vector.tensor_mul(
                            attnT[0:64, cb:cb + 512], oT[0:64, :], bcs)
                    else:
                        nc.vector.tensor_mul(
                            stg_t[:, half * 512:(half + 1) * 512],
                            oT[0:64, :], bcs)

                # head pairs (even, odd) interleaved: two independent chains
                # keep every engine busy through the other's sem latencies
                for kv in range(HKV):
                    for gp in range(2):
                        hA = kv * G + 2 * gp
                        hB = hA + 1
                        cc = hA // 2
                        slabA = slabp.tile([128, NKC * SLOT], BF16, tag="slab")
                        slabB = slabp.tile([128, NKC * SLOT], BF16, tag="slab")
                        stg_t = stO.tile([64, 2 * 512], BF16, tag="so")
                        for kc in range(NKC):
                            score_chunk(hA, kv, kc, slabA)
                            score_chunk(hB, kv, kc, slabB)
                        oTA0 = pv(hA, kv, 0, slabA)
                        oTB0 = pv(hB, kv, 0, slabB)
                        norm(hA, 0, oTA0, None)
                        norm(hB, 0, oTB0, stg_t)
                        oTA1 = pv(hA, kv, 1, slabA)
                        oTB1 = pv(hB, kv, 1, slabB)
                        norm(hA, 1, oTA1, None)
                        norm(hB, 1, oTB1, stg_t)
                        nc.gpsimd.dma_start(
                            attnT[64:128, cc * RT:(cc + 1) * RT], stg_t)
                        if DEBUG and hA == 0:
                            nc.sync.dma_start(dslab[:, :], slabA)

                if DEBUG:
                    nc.sync.dma_start(dattnT[:, :], attnT[:, :])

            # ---------------- Phase O: output projection ----------------
            with tc.tile_pool(name="wop", bufs=3) as wop, \
                 tc.tile_pool(name="obp", bufs=2) as obp, \
                 tc.tile_pool(name="psF", bufs=1, space="PSUM") as psFp:
                for ec in range(2):
                    psF = psFp.tile([128, 8 * 512], F32, tag="fps")
                    for cc in range(8):
                        woc = wop.tile([128, 512], BF16, tag="wo")
                        nc.sync.dma_start(
                            woc, wo[cc * 128:(cc + 1) * 128, ec * 512:(ec + 1) * 512])
                        for tk in range(8):
                            nc.tensor.matmul(
                                psF[:, tk * 512:(tk + 1) * 512],
                                lhsT=attnT[:, cc * RT + tk * 128:cc * RT + (tk + 1) * 128],
                                rhs=woc,
                                start=(cc == 0), stop=(cc == 7),
                                skip_group_check=True)
                    ob = obp.tile([128, 8 * 512], F32, tag="ob")
                    for tk in range(8):
                        nc.any.tensor_copy(ob[:, tk * 512:(tk + 1) * 512],
                                           psF[:, tk * 512:(tk + 1) * 512])
                        if tk % 2 == 1:  # store per 2-tk chunk for early overlap
                            nc.scalar.dma_start(
                                _dram_ap(out[:, :], (tk - 1) * 128 * C + ec * 512,
                                         [[C, 128], [128 * C, 2], [1, 512]]),
                                _ap(ob[:, 0:1], (tk - 1) * 512,
                                    [[512, 2], [1, 512]]))

    nc.compile()
    return nc


_NC = None


def _host_inputs(x, wqkv, wo):
    slopes = alibi_slopes(H)  # head h = kv*G + g matches slopes.reshape(HKV, G)

    wqkv_s = np.array(wqkv, dtype=np.float32, copy=True)
    wqkv_s[:, :C] *= SCALE  # exact power-of-two fold of the score scale into wq

    wo_bf = np.asarray(wo, dtype=np.float32).astype(ml_dtypes.bfloat16)

    j = np.arange(RT, dtype=np.float32)
    qaug = np.empty((H, 3, RT), dtype=np.float32)
    for h in range(H):
        qaug[h, 0] = -slopes[h] * (j + 512.0)
        qaug[h, 1] = slopes[h]
        qaug[h, 2] = 1.0

    i = np.arange(KR, dtype=np.float32)
    kaug_base = np.empty((3, KR), dtype=np.float32)
    kaug_base[0] = 1.0
    kaug_base[1] = i
    kaug_base[2] = 0.0

    # 0/1 multiplicative triangle masks (bf16):
    # left strip: invalid (0) where col < part; right strip: invalid where col >= part
    r = np.arange(128)[:, None]
    l = np.arange(128)[None, :]
    wnegT = np.ascontiguousarray(
        np.where(l < r, np.float32(NEG), np.float32(0.0)).astype(np.float32).T)
    ident = np.eye(128, dtype=np.float32)
    w1a = (l >= r).astype(np.float32)
    w1b = (l < r).astype(np.float32)
    w1ab = np.concatenate([w1a, w1b], axis=1).astype(ml_dtypes.bfloat16)

    in_maps = []
    for core in range(NCORES):
        b, qq = core // 4, core % 4
        t0 = qq * RT
        xsl = np.zeros((KR, C), dtype=np.float32)
        lo = t0 - W
        if lo < 0:
            xsl[-lo:, :] = x[b, 0:t0 + RT, :]
        else:
            xsl[:, :] = x[b, lo:t0 + RT, :]
        xsl = np.ascontiguousarray(xsl.T)
        kaug = kaug_base.copy()
        if lo < 0:
            kaug[2, :W] = NEG  # left-edge penalty kills padded keys
        kaug4 = np.ascontiguousarray(np.tile(kaug, (1, HKV)))
        cns = np.zeros((3, 512), dtype=np.float32)
        cns[0] = 1.0
        in_maps.append(dict(xs=xsl, wqkv=wqkv_s, wo=wo_bf,
                            qaug=qaug, kaug=kaug4, wnegT=wnegT, ident=ident,
                            w1ab=w1ab, cns=cns))
    return in_maps


def kernel(x, wqkv, wo):
    global _NC
    if _NC is None:
        _NC = build_nc()
    in_maps = _host_inputs(np.asarray(x), np.asarray(wqkv), np.asarray(wo))
    res = run_bass_kernel_spmd(_NC, in_maps, list(range(NCORES)))
    full = np.empty((B, T, C), dtype=np.float32)
    for core in range(NCORES):
        b, qq = core // 4, core % 4
        full[b, qq * RT:(qq + 1) * RT, :] = res.results[core]["out"]
    return full


# revision 51
# speedup vs baseline: 3.2096x; 3.2096x over previous
"""Sliding-window GQA causal self-attention (ALiBi) Trainium2 Bass kernel.

Problem: B=2, T=4096, C=1024, H=16, HKV=4 (GQA G=4), D=64, window W=512,
fused qkv projection + sliding-window attention + output projection.

Sharding: data-parallel over (batch x T/4) -> 8 cores. Each core computes
1024 query rows of one batch plus a 512-row k/v halo. No collectives.

Per-core dataflow (q/k in float32r; p/v/attnT/wo in bf16):
  - x arrives host-transposed; xT streamed in 512-column time slices
  - qT/kT computed transposed (stationary wqkv chunk, streaming xT);
    v computed natural and stored as 65-wide blocks (64 dims + ones col)
  - scores computed TRANSPOSED per 128-row k-chunk: sT[k, q] via
    lhsT=kT chunk, rhs=qT window; 3 augmentation rows fold in the ALiBi
    bias (rank-2 in absolute coords) and the left-edge -1e9 penalty
  - exp (no max-subtraction: scores are N(0,~6.5), exp cannot overflow
    at <13 sigma; softmax shift-invariance keeps this exact) writes the
    bf16 p slab directly in PV layout -- no PE transposes needed
  - window mask applied multiplicatively (0/1 triangles) on the bf16 slab
  - PV: per q-half, variable-width accumulation over 8 k-chunks covering
    only the valid band; ones-column of v gives the row-sum for free
  - normalize: reciprocal row -> PE broadcast [64,512] -> one DVE multiply
    that writes attnT (even heads) or a staging tile DMA'd to the upper
    64 partitions (odd heads)
  - out = attnT.T @ wo with wo streamed in bf16 chunks
"""

import math
from contextlib import ExitStack

import numpy as np
import ml_dtypes

import concourse.bass as bass
from concourse import bacc
import concourse.mybir as mybir
import concourse.tile as tile
from concourse.bass_utils import run_bass_kernel_spmd

F32 = mybir.dt.float32
F32R = mybir.dt.float32r
BF16 = mybir.dt.bfloat16

B, T, C = 2, 4096, 1024
H, HKV, G, D = 16, 4, 4, 64
W = 512
NCORES = 8
RT = 1024              # own query rows per core
KR = RT + W            # k/v slab rows (512 halo + 1024 own)
NKC = KR // 128        # 12 k-chunks of 128
SCALE = D ** -0.5      # 0.125, exact power of two
NEG = -1e9
KCOL0 = C              # wqkv col offset of k
VCOL0 = C + HKV * D    # wqkv col offset of v
VB = 65                # v block width (64 dims + ones column)
SLOT = 640             # p-slab slot width per k-chunk (wcol coords)

Exp = mybir.ActivationFunctionType.Exp

DEBUG = False


def alibi_slopes(n_head: int) -> np.ndarray:
    def slopes_power_of_2(n):
        start = 2.0 ** (-(2.0 ** (-(math.log2(n) - 3))))
        return [start * start ** i for i in range(n)]

    if float(math.log2(n_head)).is_integer():
        s = slopes_power_of_2(n_head)
    else:
        closest = 2 ** math.floor(math.log2(n_head))
        s = slopes_power_of_2(closest)
        s2 = slopes_power_of_2(2 * closest)
        s += s2[0::2][: n_head - closest]
    return np.array(s, dtype=np.float32)


def _ap(base, extra_off, dims):
    """AP with the partition dim of `base` and custom free dims."""
    return bass.AP(tensor=base.tensor, offset=base.offset + extra_off,
                   ap=[list(base.ap[0])] + dims)


def _dram_ap(t, off, dims):
    return bass.AP(tensor=t.tensor, offset=t.offset + off, ap=dims)


# per-chunk written window in wcol coords: [woff, woff+wlen)
def chunk_window(kc):
    wlo = max(0, 128 * kc - 512)          # q range [wlo, whi)
    whi = min(RT, 128 * kc + 128)
    woff = wlo - (128 * kc - 512)         # = max(0, 512 - 128*kc)
    return wlo, whi, woff, whi - wlo


def build_nc(loop: int = 1) -> bacc.Bacc:
    nc = bacc.Bacc("TRN2", target_bir_lowering=False)

    xs = nc.dram_tensor("xs", [C, KR], F32R, kind="ExternalInput")  # host-transposed
    wqkv = nc.dram_tensor("wqkv", [C, C + 2 * HKV * D], F32R, kind="ExternalInput")
    wo = nc.dram_tensor("wo", [C, C], BF16, kind="ExternalInput")
    qaug = nc.dram_tensor("qaug", [H, 3, RT], F32R, kind="ExternalInput")
    kaug = nc.dram_tensor("kaug", [3, HKV * KR], F32R, kind="ExternalInput")
    wnegT = nc.dram_tensor("wnegT", [128, 128], F32R, kind="ExternalInput")
    ident = nc.dram_tensor("ident", [128, 128], F32R, kind="ExternalInput")
    w1ab = nc.dram_tensor("w1ab", [128, 256], BF16, kind="ExternalInput")
    cns = nc.dram_tensor("cns", [3, 512], F32R, kind="ExternalInput")
    out = nc.dram_tensor("out", [RT, C], F32, kind="ExternalOutput")
    if DEBUG:
        dqT = nc.dram_tensor("dqT", [128, H * RT], F32, kind="ExternalOutput")
        dkT = nc.dram_tensor("dkT", [128, HKV * KR], F32, kind="ExternalOutput")
        dvsl = nc.dram_tensor("dvsl", [128, NKC * HKV * VB], BF16, kind="ExternalOutput")
        dslab = nc.dram_tensor("dslab", [128, NKC * SLOT], BF16, kind="ExternalOutput")
        dattnT = nc.dram_tensor("dattnT", [128, 8 * RT], BF16, kind="ExternalOutput")
        drecs = nc.dram_tensor("drecs", [2, 512], F32, kind="ExternalOutput")

    with tile.TileContext(nc) as tc:
      for _rep in range(loop):
        with ExitStack() as ctx:
            persist = ctx.enter_context(tc.tile_pool(name="persist", bufs=1))

            qT = persist.tile([128, H * RT], F32R)      # [0:64] data, [64:67] aug
            kT = persist.tile([128, HKV * KR], F32R)    # [0:64] data, [64:67] aug
            vsl = persist.tile([128, NKC * HKV * VB], BF16)  # v natural, kc-major
            attnT = persist.tile([128, 8 * RT], BF16)   # [c in chunk, cc*RT + t]
            wnegT_sb = persist.tile([128, 128], F32R)   # -1e9 left tri^T
            id_sb = persist.tile([128, 128], F32R)
            w1ab_sb = persist.tile([128, 256], BF16)    # [0/1 left | right] tri
            # broadcast weights: row 64 = 1, row 65 = 0 (contract-2 bcast mm).
            # Engine ops need partition base % 32 == 0: zero an aligned range
            # first, then overwrite row 64.
            bwt = persist.tile([128, 64], F32R)
            nc.gpsimd.dma_start(bwt[64:66, 0:64],
                              _dram_ap(cns[0:2, 0:64], 0, [[512, 2], [1, 64]]))
            # two recs tiles, row 65 zeroed once (killed by 0-weight in bcast)
            recsA = persist.tile([128, 512], F32R)
            recsB = persist.tile([128, 512], F32R)
            recsT = [recsA, recsB]
            nc.gpsimd.dma_start(recsA[65:66, :], cns[1:2, :])
            nc.gpsimd.dma_start(recsB[65:66, :], cns[2:3, :])

            # aux loads on the Pool (SWDGE) queue; SP queue stays clear for
            # the critical-path xs/wv loads
            nc.gpsimd.dma_start(wnegT_sb, wnegT[:, :])
            nc.gpsimd.dma_start(id_sb, ident[:, :])
            nc.gpsimd.dma_start(w1ab_sb, w1ab[:, :])
            # qaug: dst rows 64:67 of qT over all heads, one DMA
            # (src/dst dim structures must match: both (3, H, RT))
            nc.gpsimd.dma_start(
                _ap(qT[64:67, 0:1], 0, [[RT, H], [1, RT]]),
                _dram_ap(qaug[0, :, :], 0, [[RT, 3], [3 * RT, H], [1, RT]]))
            # kaug: pre-tiled [3, 4*KR] on host, one DMA
            nc.gpsimd.dma_start(kT[64:67, 0:HKV * KR], kaug[:, :])
            # ones columns of vsl (col 64 of each 65-wide block)
            for kcc in range(NKC):
                base = vsl[:, 0:1]
                nc.vector.memset(
                    _ap(base, kcc * HKV * VB + 64, [[VB, HKV], [1, 1]]), 1.0)

            # ---------------- Phase Q: qkv projection ----------------
            with tc.tile_pool(name="xTp", bufs=3) as xTp, \
                 tc.tile_pool(name="stg", bufs=2) as stg, \
                 tc.tile_pool(name="wqp", bufs=2) as wqp, \
                 tc.tile_pool(name="wvp", bufs=1) as wvp, \
                 tc.tile_pool(name="psQK", bufs=4, space="PSUM") as psQK, \
                 tc.tile_pool(name="psV", bufs=3, space="PSUM") as psV:

                wv = wvp.tile([128, 8 * 256], F32R)

                xTts = {}

                def slice_dma(ts, first_tki_only=False, rest=False):
                    if rest:
                        xTt = xTts[ts]
                    else:
                        xTt = xTp.tile([128, 8 * 512], F32R, tag="xts")
                        xTts[ts] = xTt
                    # one DMA per 128-col tki chunk so compute starts early
                    tkis = ([0] if first_tki_only else
                            ([1, 2, 3] if rest else [0, 1, 2, 3]))
                    for tki in tkis:
                        nc.sync.dma_start(
                            _ap(xTt[:, 0:1], tki * 128, [[512, 8], [1, 128]]),
                            _dram_ap(xs[:, :], ts * 512 + tki * 128,
                                     [[KR, 128], [128 * KR, 8], [1, 128]]))

                def slice_v(ts):
                    xTt = xTts[ts]
                    for tki in range(4):
                        tk = ts * 4 + tki
                        psv = psV.tile([128, 256], F32, tag="vps")
                        for cc in range(8):
                            nc.tensor.matmul(
                                psv,
                                lhsT=xTt[:, cc * 512 + tki * 128:cc * 512 + (tki + 1) * 128],
                                rhs=wv[:, cc * 256:(cc + 1) * 256],
                                start=(cc == 0), stop=(cc == 7))
                        # scatter 4 kv blocks into 65-wide slots (bf16)
                        nc.vector.tensor_copy(
                            _ap(vsl[:, 0:1], tk * HKV * VB, [[VB, HKV], [1, 64]]),
                            psv.rearrange("p (a b) -> p a b", b=64))

                def load_wq(fc2):
                    # fc2 0..3: q feature pairs; fc2 4: k features (both kv pairs)
                    fcol = fc2 * 256 if fc2 < 4 else KCOL0
                    wq = wqp.tile([128, 8 * 256], F32R, tag="wqf")
                    nc.sync.dma_start(
                        _ap(wq[:, 0:1], 0, [[256, 8], [1, 256]]),
                        _dram_ap(wqkv[:, :], fcol,
                                 [[C + 512, 128], [128 * (C + 512), 8], [1, 256]]))
                    return wq

                def qk_slab(fc2, ts_list, wq=None):
                    if wq is None:
                        wq = load_wq(fc2)
                    for ts in ts_list:
                        for fi in range(2):
                            fc = fc2 * 2 + fi
                            ps = psQK.tile([128, 512], F32, tag="qkps")
                            for cc in range(8):
                                nc.tensor.matmul(
                                    ps,
                                    lhsT=wq[:, cc * 256 + fi * 128:cc * 256 + (fi + 1) * 128],
                                    rhs=xTts[ts][:, cc * 512:(cc + 1) * 512],
                                    start=(cc == 0), stop=(cc == 7))
                            st = stg.tile([128, 512], F32R, tag="stg")
                            nc.scalar.copy(st[64:128, :], ps[64:128, :])
                            if fc2 < 4:
                                h0, h1 = 2 * fc, 2 * fc + 1
                                toff = (ts - 1) * 512
                                nc.vector.tensor_copy(
                                    qT[0:64, h0 * RT + toff:h0 * RT + toff + 512],
                                    ps[0:64, :])
                                nc.scalar.dma_start(
                                    qT[0:64, h1 * RT + toff:h1 * RT + toff + 512],
                                    st[64:128, :])
                            else:
                                kv0, kv1 = 2 * fi, 2 * fi + 1
                                toff = ts * 512
                                nc.vector.tensor_copy(
                                    kT[0:64, kv0 * KR + toff:kv0 * KR + toff + 512],
                                    ps[0:64, :])
                                nc.scalar.dma_start(
                                    kT[0:64, kv1 * KR + toff:kv1 * KR + toff + 512],
                                    st[64:128, :])

                slice_dma(0, first_tki_only=True)
                nc.sync.dma_start(
                    _ap(wv[:, 0:1], 0, [[256, 8], [1, 256]]),
                    _dram_ap(wqkv[:, :], VCOL0,
                             [[C + 512, 128], [128 * (C + 512), 8], [1, 256]]))
                slice_dma(0, rest=True)
                wqk = load_wq(4)
                slice_dma(1)
                slice_v(0)
                qk_slab(4, [0], wqk)
                slice_v(1)
                slice_dma(2)
                qk_slab(4, [1], wqk)
                slice_v(2)
                qk_slab(4, [2], wqk)
                for fc2 in range(4):
                    qk_slab(fc2, [1, 2])

            # wo prefetch pool: loads issue on the idle SP queue during
            # attention so phase O starts with data ready
            wop = ctx.enter_context(tc.tile_pool(name="wop", bufs=3))
            wo_pre = {}

            def load_wo(ec, cc):
                woc = wop.tile([128, 512], BF16, tag="wo")
                nc.sync.dma_start(
                    woc, wo[cc * 128:(cc + 1) * 128, ec * 512:(ec + 1) * 512])
                return woc

            for _cc in range(2):
                wo_pre[(0, _cc)] = load_wo(0, _cc)

            # ---------------- Phase A: attention ----------------
            with tc.tile_pool(name="slabp", bufs=3) as slabp, \
                 tc.tile_pool(name="stO", bufs=2) as stO, \
                 tc.tile_pool(name="bcsp", bufs=2) as bcsp, \
                 tc.tile_pool(name="psOB", bufs=3, space="PSUM") as psOB, \
                 tc.tile_pool(name="psB", bufs=1, space="PSUM") as psB, \
                 tc.tile_pool(name="psS", bufs=2, space="PSUM") as psS:

                if DEBUG:
                    nc.sync.dma_start(dqT[:, :], qT[:, :].bitcast(F32))
                    nc.sync.dma_start(dkT[:, :], kT[:, :].bitcast(F32))
                    nc.sync.dma_start(dvsl[:, :], vsl[:, :])

                def score_chunk(h, kv, kc, slab):
                    wlo, whi, woff, wlen = chunk_window(kc)
                    stile = psS.tile([128, SLOT], F32, tag="sc")
                    kstat = kT[0:67, kv * KR + kc * 128:kv * KR + (kc + 1) * 128]
                    w0 = min(wlen, 512)
                    ltri = kc >= 4   # left tri: future keys
                    nc.tensor.matmul(
                        stile[:, 0:w0], lhsT=kstat,
                        rhs=qT[0:67, h * RT + wlo:h * RT + wlo + w0],
                        start=True, stop=not ltri,
                        skip_group_check=True)
                    if wlen > 512:
                        nc.tensor.matmul(
                            stile[:, 512:wlen], lhsT=kstat,
                            rhs=qT[0:67, h * RT + wlo + 512:h * RT + whi],
                            start=True, stop=True,
                            skip_group_check=True)
                    # Left tri = FUTURE keys (ALiBi bias positive, exp would
                    # overflow): -1e9 accumulated via PE (lhsT = tri^T,
                    # rhs = I) -> exp gives exact zeros.
                    if ltri:
                        nc.tensor.matmul(
                            stile[:, 0:128],
                            lhsT=wnegT_sb[:, 0:128], rhs=id_sb,
                            start=False, stop=True,
                            skip_group_check=True)
                    # No max-subtraction: valid scores are N(0, ~6.5);
                    # exp overflow needs ~13 sigma; bf16 range is ample.
                    nc.scalar.activation(
                        slab[:, kc * SLOT + woff:kc * SLOT + woff + wlen],
                        stile[:, 0:wlen], Exp, bias=0.0)
                    # Right tri = beyond-window past keys (finite): zero
                    # multiplicatively on the idle Pool engine.
                    if kc <= 7:
                        tri = slab[:, kc * SLOT + 512:kc * SLOT + 640]
                        nc.gpsimd.tensor_mul(tri, tri, w1ab_sb[:, 128:256])

                def pv(h, kv, half, slab):
                    oT = psOB.tile([VB, 512], F32, tag="ps_ob")
                    # s=3 first: full 512-wide, zeroes the bank
                    order = [3, 0, 1, 2, 4, 5, 6, 7]
                    for i, s in enumerate(order):
                        kc = 4 * half + s
                        c0 = max(0, 128 * s - 512)
                        c1 = min(512, 128 * s + 128)
                        rw0 = max(512 - 128 * s, 0)
                        nc.tensor.matmul(
                            oT[:, c0:c1],
                            lhsT=vsl[:, kc * HKV * VB + kv * VB:
                                     kc * HKV * VB + (kv + 1) * VB],
                            rhs=slab[:, kc * SLOT + rw0:kc * SLOT + rw0 + (c1 - c0)],
                            start=(i == 0), stop=(i == 7),
                            skip_group_check=True)
                    return oT

                _rec_i = [0]

                def norm(h, half, oT, stg_t):
                    cc = h // 2
                    recs = recsT[_rec_i[0] % 2]
                    _rec_i[0] += 1
                    with nc.allow_low_precision(reason="recip rounded to f32r"):
                        nc.vector.reciprocal(recs[64:65, :], oT[64:65, :])
                    bc = psB.tile([64, 512], F32, tag="bc")
                    nc.tensor.matmul(bc, lhsT=bwt[64:66, 0:64],
                                     rhs=recs[64:66, :],
                                     start=True, stop=True)
                    # TT can take at most one PSUM operand: stage bc
                    bcs = bcsp.tile([64, 512], F32, tag="bcs")
                    nc.vector.tensor_copy(bcs, bc)
                    if DEBUG and h == 0:
                        nc.sync.dma_start(
                            drecs[half:half + 1, :], recs[64:65, :].bitcast(F32))
                    cb = cc * RT + half * 512
                    if h % 2 == 0:
                        nc.vector.tensor_mul(
                            attnT[0:64, cb:cb + 512], oT[0:64, :], bcs)
                    else:
                        nc.vector.tensor_mul(
                            stg_t[:, half * 512:(half + 1) * 512],
                            oT[0:64, :], bcs)

                # head pairs (even, odd) interleaved: two independent chains
                # keep every engine busy through the other's sem latencies
                for kv in range(HKV):
                    for gp in range(2):
                        hA = kv * G + 2 * gp
                        hB = hA + 1
                        cc = hA // 2
                        slabA = slabp.tile([128, NKC * SLOT], BF16, tag="slab")
                        slabB = slabp.tile([128, NKC * SLOT], BF16, tag="slab")
                        stg_t = stO.tile([64, 2 * 512], BF16, tag="so")
                        for kc in range(NKC):
                            score_chunk(hA, kv, kc, slabA)
                            score_chunk(hB, kv, kc, slabB)
                        oTA0 = pv(hA, kv, 0, slabA)
                        oTB0 = pv(hB, kv, 0, slabB)
                        norm(hA, 0, oTA0, None)
                        norm(hB, 0, oTB0, stg_t)
                        oTA1 = pv(hA, kv, 1, slabA)
                        oTB1 = pv(hB, kv, 1, slabB)
                        norm(hA, 1, oTA1, None)
                        norm(hB, 1, oTB1, stg_t)
                        nc.gpsimd.dma_start(
                            attnT[64:128, cc * RT:(cc + 1) * RT], stg_t)
                        if DEBUG and hA == 0:
                            nc.sync.dma_start(dslab[:, :], slabA)

                if DEBUG:
                    nc.sync.dma_start(dattnT[:, :], attnT[:, :])

            # ---------------- Phase O: output projection ----------------
            with tc.tile_pool(name="obp", bufs=2) as obp, \
                 tc.tile_pool(name="psF", bufs=1, space="PSUM") as psFp:
                for ec in range(2):
                    psF = psFp.tile([128, 8 * 512], F32, tag="fps")
                    for cc in range(8):
                        woc = wo_pre.pop((ec, cc), None)
                        if woc is None:
                            woc = load_wo(ec, cc)
                        for tk in range(8):
                            nc.tensor.matmul(
                                psF[:, tk * 512:(tk + 1) * 512],
                                lhsT=attnT[:, cc * RT + tk * 128:cc * RT + (tk + 1) * 128],
                                rhs=woc,
                                start=(cc == 0), stop=(cc == 7),
                                skip_group_check=True)
                    ob = obp.tile([128, 8 * 512], F32, tag="ob")
                    for tk in range(8):
                        nc.any.tensor_copy(ob[:, tk * 512:(tk + 1) * 512],
                                           psF[:, tk * 512:(tk + 1) * 512])
                        if tk % 2 == 1:  # store per 2-tk chunk for early overlap
                            nc.scalar.dma_start(
                                _dram_ap(out[:, :], (tk - 1) * 128 * C + ec * 512,
                                         [[C, 128], [128 * C, 2], [1, 512]]),
                                _ap(ob[:, 0:1], (tk - 1) * 512,
                                    [[512, 2], [1, 512]]))

    nc.compile()
    return nc


_NC = None


def _host_inputs(x, wqkv, wo):
    slopes = alibi_slopes(H)  # head h = kv*G + g matches slopes.reshape(HKV, G)

    wqkv_s = np.array(wqkv, dtype=np.float32, copy=True)
    wqkv_s[:, :C] *= SCALE  # exact power-of-two fold of the score scale into wq

    wo_bf = np.asarray(wo, dtype=np.float32).astype(ml_dtypes.bfloat16)

    j = np.arange(RT, dtype=np.float32)
    qaug = np.empty((H, 3, RT), dtype=np.float32)
    for h in range(H):
        qaug[h, 0] = -slopes[h] * (j + 512.0)
        qaug[h, 1] = slopes[h]
        qaug[h, 2] = 1.0

    i = np.arange(KR, dtype=np.float32)
    kaug_base = np.empty((3, KR), dtype=np.float32)
    kaug_base[0] = 1.0
    kaug_base[1] = i
    kaug_base[2] = 0.0

    # 0/1 multiplicative triangle masks (bf16):
    # left strip: invalid (0) where col < part; right strip: invalid where col >= part
    r = np.arange(128)[:, None]
    l = np.arange(128)[None, :]
    wnegT = np.ascontiguousarray(
        np.where(l < r, np.float32(NEG), np.float32(0.0)).astype(np.float32).T)
    ident = np.eye(128, dtype=np.float32)
    w1a = (l >= r).astype(np.float32)
    w1b = (l < r).astype(np.float32)
    w1ab = np.concatenate([w1a, w1b], axis=1).astype(ml_dtypes.bfloat16)

    in_maps = []
    for core in range(NCORES):
        b, qq = core // 4, core % 4
        t0 = qq * RT
        xsl = np.zeros((KR, C), dtype=np.float32)
        lo = t0 - W
        if lo < 0:
            xsl[-lo:, :] = x[b, 0:t0 + RT, :]
        else:
            xsl[:, :] = x[b, lo:t0 + RT, :]
        xsl = np.ascontiguousarray(xsl.T)
        kaug = kaug_base.copy()
        if lo < 0:
            kaug[2, :W] = NEG  # left-edge penalty kills padded keys
        kaug4 = np.ascontiguousarray(np.tile(kaug, (1, HKV)))
        cns = np.zeros((3, 512), dtype=np.float32)
        cns[0] = 1.0
        in_maps.append(dict(xs=xsl, wqkv=wqkv_s, wo=wo_bf,
                            qaug=qaug, kaug=kaug4, wnegT=wnegT, ident=ident,
                            w1ab=w1ab, cns=cns))
    return in_maps


def kernel(x, wqkv, wo):
    global _NC
    if _NC is None:
        _NC = build_nc()
    in_maps = _host_inputs(np.asarray(x), np.asarray(wqkv), np.asarray(wo))
    res = run_bass_kernel_spmd(_NC, in_maps, list(range(NCORES)))
    full = np.empty((B, T, C), dtype=np.float32)
    for core in range(NCORES):
        b, qq = core // 4, core % 4
        full[b, qq * RT:(qq + 1) * RT, :] = res.results[core]["out"]
    return full


# revision 52
# speedup vs baseline: 4.6034x; 1.4342x over previous
"""Sliding-window GQA causal self-attention (ALiBi) Trainium2 Bass kernel.

Problem: B=2, T=4096, C=1024, H=16, HKV=4 (GQA G=4), D=64, window W=512,
fused qkv projection + sliding-window attention + output projection.

Sharding: data-parallel over (batch x T/4) -> 8 cores. Each core computes
1024 query rows of one batch plus a 512-row k/v halo. No collectives.

Per-core dataflow (q/k in float32r; p/v/attnT/wo in bf16):
  - x arrives host-transposed; xT streamed in 512-column time slices
  - qT/kT computed transposed (stationary wqkv chunk, streaming xT);
    v computed natural and stored as 65-wide blocks (64 dims + ones col)
  - scores computed TRANSPOSED per 128-row k-chunk: sT[k, q] via
    lhsT=kT chunk, rhs=qT window; 3 augmentation rows fold in the ALiBi
    bias (rank-2 in absolute coords) and the left-edge -1e9 penalty
  - exp (no max-subtraction: scores are N(0,~6.5), exp cannot overflow
    at <13 sigma; softmax shift-invariance keeps this exact) writes the
    bf16 p slab directly in PV layout -- no PE transposes needed
  - window mask applied multiplicatively (0/1 triangles) on the bf16 slab
  - PV: per q-half, variable-width accumulation over 8 k-chunks covering
    only the valid band; ones-column of v gives the row-sum for free
  - normalize: reciprocal row -> PE broadcast [64,512] -> one DVE multiply
    that writes attnT (even heads) or a staging tile DMA'd to the upper
    64 partitions (odd heads)
  - out = attnT.T @ wo with wo streamed in bf16 chunks
"""

import math
from contextlib import ExitStack

import numpy as np
import ml_dtypes

import concourse.bass as bass
from concourse import bacc
import concourse.mybir as mybir
import concourse.tile as tile
from concourse.bass_utils import run_bass_kernel_spmd

F32 = mybir.dt.float32
F32R = mybir.dt.float32r
BF16 = mybir.dt.bfloat16

B, T, C = 2, 4096, 1024
H, HKV, G, D = 16, 4, 4, 64
W = 512
NCORES = 8
RT = 1024              # own query rows per core
KR = RT + W            # k/v slab rows (512 halo + 1024 own)
NKC = KR // 128        # 12 k-chunks of 128
SCALE = D ** -0.5      # 0.125, exact power of two
NEG = -1e9
KCOL0 = C              # wqkv col offset of k
VCOL0 = C + HKV * D    # wqkv col offset of v
VB = 65                # v block width (64 dims + ones column)
SLOT = 640             # p-slab slot width per k-chunk (wcol coords)

Exp = mybir.ActivationFunctionType.Exp

DEBUG = False


def alibi_slopes(n_head: int) -> np.ndarray:
    def slopes_power_of_2(n):
        start = 2.0 ** (-(2.0 ** (-(math.log2(n) - 3))))
        return [start * start ** i for i in range(n)]

    if float(math.log2(n_head)).is_integer():
        s = slopes_power_of_2(n_head)
    else:
        closest = 2 ** math.floor(math.log2(n_head))
        s = slopes_power_of_2(closest)
        s2 = slopes_power_of_2(2 * closest)
        s += s2[0::2][: n_head - closest]
    return np.array(s, dtype=np.float32)


def _ap(base, extra_off, dims):
    """AP with the partition dim of `base` and custom free dims."""
    return bass.AP(tensor=base.tensor, offset=base.offset + extra_off,
                   ap=[list(base.ap[0])] + dims)


def _dram_ap(t, off, dims):
    return bass.AP(tensor=t.tensor, offset=t.offset + off, ap=dims)


# per-chunk written window in wcol coords: [woff, woff+wlen)
def chunk_window(kc):
    wlo = max(0, 128 * kc - 512)          # q range [wlo, whi)
    whi = min(RT, 128 * kc + 128)
    woff = wlo - (128 * kc - 512)         # = max(0, 512 - 128*kc)
    return wlo, whi, woff, whi - wlo


def build_nc(loop: int = 1) -> bacc.Bacc:
    nc = bacc.Bacc("TRN2", target_bir_lowering=False)

    xs = nc.dram_tensor("xs", [C, KR], F32R, kind="ExternalInput")  # host-transposed
    wqkv = nc.dram_tensor("wqkv", [C, C + 2 * HKV * D], F32R, kind="ExternalInput")
    wo = nc.dram_tensor("wo", [C, C], BF16, kind="ExternalInput")
    qaug = nc.dram_tensor("qaug", [H, 3, RT], F32R, kind="ExternalInput")
    kaug = nc.dram_tensor("kaug", [3, HKV * KR], F32R, kind="ExternalInput")
    wnegT = nc.dram_tensor("wnegT", [128, 256], F32R, kind="ExternalInput")
    ident = nc.dram_tensor("ident", [128, 128], F32R, kind="ExternalInput")
    cns = nc.dram_tensor("cns", [3, 512], F32R, kind="ExternalInput")
    out = nc.dram_tensor("out", [RT, C], F32, kind="ExternalOutput")
    if DEBUG:
        dqT = nc.dram_tensor("dqT", [128, H * RT], F32, kind="ExternalOutput")
        dkT = nc.dram_tensor("dkT", [128, HKV * KR], F32, kind="ExternalOutput")
        dvsl = nc.dram_tensor("dvsl", [128, NKC * HKV * VB], BF16, kind="ExternalOutput")
        dslab = nc.dram_tensor("dslab", [128, NKC * SLOT], BF16, kind="ExternalOutput")
        dattnT = nc.dram_tensor("dattnT", [128, 8 * RT], BF16, kind="ExternalOutput")
        drecs = nc.dram_tensor("drecs", [2, 512], F32, kind="ExternalOutput")

    with tile.TileContext(nc) as tc:
      for _rep in range(loop):
        with ExitStack() as ctx:
            persist = ctx.enter_context(tc.tile_pool(name="persist", bufs=1))

            qT = persist.tile([128, H * RT], F32R)      # [0:64] data, [64:67] aug
            kT = persist.tile([128, HKV * KR], F32R)    # [0:64] data, [64:67] aug
            vsl = persist.tile([128, NKC * HKV * VB], BF16)  # v natural, kc-major
            attnT = persist.tile([128, 8 * RT], BF16)   # [c in chunk, cc*RT + t]
            wnegT_sb = persist.tile([128, 256], F32R)   # [-1e9 left | right] tri^T
            id_sb = persist.tile([128, 128], F32R)
            # broadcast weights: row 64 = 1, row 65 = 0 (contract-2 bcast mm).
            # Engine ops need partition base % 32 == 0: zero an aligned range
            # first, then overwrite row 64.
            bwt = persist.tile([128, 64], F32R)
            nc.gpsimd.dma_start(bwt[64:66, 0:64],
                              _dram_ap(cns[0:2, 0:64], 0, [[512, 2], [1, 64]]))
            # two recs tiles, row 65 zeroed once (killed by 0-weight in bcast)
            recsA = persist.tile([128, 512], F32R)
            recsB = persist.tile([128, 512], F32R)
            recsT = [recsA, recsB]
            nc.gpsimd.dma_start(recsA[65:66, :], cns[1:2, :])
            nc.gpsimd.dma_start(recsB[65:66, :], cns[2:3, :])

            # aux loads on the Pool (SWDGE) queue; SP queue stays clear for
            # the critical-path xs/wv loads
            nc.gpsimd.dma_start(wnegT_sb, wnegT[:, :])
            nc.gpsimd.dma_start(id_sb, ident[:, :])
            # qaug: dst rows 64:67 of qT over all heads, one DMA
            # (src/dst dim structures must match: both (3, H, RT))
            nc.gpsimd.dma_start(
                _ap(qT[64:67, 0:1], 0, [[RT, H], [1, RT]]),
                _dram_ap(qaug[0, :, :], 0, [[RT, 3], [3 * RT, H], [1, RT]]))
            # kaug: pre-tiled [3, 4*KR] on host, one DMA
            nc.gpsimd.dma_start(kT[64:67, 0:HKV * KR], kaug[:, :])
            # ones columns of vsl (col 64 of each 65-wide block)
            for kcc in range(NKC):
                base = vsl[:, 0:1]
                nc.vector.memset(
                    _ap(base, kcc * HKV * VB + 64, [[VB, HKV], [1, 1]]), 1.0)

            # ---------------- Phase Q: qkv projection ----------------
            with tc.tile_pool(name="xTp", bufs=3) as xTp, \
                 tc.tile_pool(name="stg", bufs=2) as stg, \
                 tc.tile_pool(name="wqp", bufs=2) as wqp, \
                 tc.tile_pool(name="wvp", bufs=1) as wvp, \
                 tc.tile_pool(name="psQK", bufs=4, space="PSUM") as psQK, \
                 tc.tile_pool(name="psV", bufs=3, space="PSUM") as psV:

                wv = wvp.tile([128, 8 * 256], F32R)

                xTts = {}

                def slice_dma(ts, first_tki_only=False, rest=False):
                    if rest:
                        xTt = xTts[ts]
                    else:
                        xTt = xTp.tile([128, 8 * 512], F32R, tag="xts")
                        xTts[ts] = xTt
                    # one DMA per 128-col tki chunk so compute starts early
                    tkis = ([0] if first_tki_only else
                            ([1, 2, 3] if rest else [0, 1, 2, 3]))
                    for tki in tkis:
                        nc.sync.dma_start(
                            _ap(xTt[:, 0:1], tki * 128, [[512, 8], [1, 128]]),
                            _dram_ap(xs[:, :], ts * 512 + tki * 128,
                                     [[KR, 128], [128 * KR, 8], [1, 128]]))

                def slice_v(ts):
                    xTt = xTts[ts]
                    for tki in range(4):
                        tk = ts * 4 + tki
                        psv = psV.tile([128, 256], F32, tag="vps")
                        for cc in range(8):
                            nc.tensor.matmul(
                                psv,
                                lhsT=xTt[:, cc * 512 + tki * 128:cc * 512 + (tki + 1) * 128],
                                rhs=wv[:, cc * 256:(cc + 1) * 256],
                                start=(cc == 0), stop=(cc == 7))
                        # scatter 4 kv blocks into 65-wide slots (bf16)
                        nc.vector.tensor_copy(
                            _ap(vsl[:, 0:1], tk * HKV * VB, [[VB, HKV], [1, 64]]),
                            psv.rearrange("p (a b) -> p a b", b=64))

                def load_wq(fc2):
                    # fc2 0..3: q feature pairs; fc2 4: k features (both kv pairs)
                    fcol = fc2 * 256 if fc2 < 4 else KCOL0
                    wq = wqp.tile([128, 8 * 256], F32R, tag="wqf")
                    nc.sync.dma_start(
                        _ap(wq[:, 0:1], 0, [[256, 8], [1, 256]]),
                        _dram_ap(wqkv[:, :], fcol,
                                 [[C + 512, 128], [128 * (C + 512), 8], [1, 256]]))
                    return wq

                def qk_slab(fc2, ts_list, wq=None):
                    if wq is None:
                        wq = load_wq(fc2)
                    for ts in ts_list:
                        for fi in range(2):
                            fc = fc2 * 2 + fi
                            ps = psQK.tile([128, 512], F32, tag="qkps")
                            for cc in range(8):
                                nc.tensor.matmul(
                                    ps,
                                    lhsT=wq[:, cc * 256 + fi * 128:cc * 256 + (fi + 1) * 128],
                                    rhs=xTts[ts][:, cc * 512:(cc + 1) * 512],
                                    start=(cc == 0), stop=(cc == 7))
                            st = stg.tile([128, 512], F32R, tag="stg")
                            nc.scalar.copy(st[64:128, :], ps[64:128, :])
                            if fc2 < 4:
                                h0, h1 = 2 * fc, 2 * fc + 1
                                toff = (ts - 1) * 512
                                nc.vector.tensor_copy(
                                    qT[0:64, h0 * RT + toff:h0 * RT + toff + 512],
                                    ps[0:64, :])
                                nc.scalar.dma_start(
                                    qT[0:64, h1 * RT + toff:h1 * RT + toff + 512],
                                    st[64:128, :])
                            else:
                                kv0, kv1 = 2 * fi, 2 * fi + 1
                                toff = ts * 512
                                nc.vector.tensor_copy(
                                    kT[0:64, kv0 * KR + toff:kv0 * KR + toff + 512],
                                    ps[0:64, :])
                                nc.scalar.dma_start(
                                    kT[0:64, kv1 * KR + toff:kv1 * KR + toff + 512],
                                    st[64:128, :])

                slice_dma(0, first_tki_only=True)
                nc.sync.dma_start(
                    _ap(wv[:, 0:1], 0, [[256, 8], [1, 256]]),
                    _dram_ap(wqkv[:, :], VCOL0,
                             [[C + 512, 128], [128 * (C + 512), 8], [1, 256]]))
                slice_dma(0, rest=True)
                wqk = load_wq(4)
                slice_dma(1)
                slice_v(0)
                qk_slab(4, [0], wqk)
                slice_v(1)
                slice_dma(2)
                qk_slab(4, [1], wqk)
                slice_v(2)
                qk_slab(4, [2], wqk)
                for fc2 in range(4):
                    qk_slab(fc2, [1, 2])

            # wo prefetch pool: loads issue on the idle SP queue during
            # attention so phase O starts with data ready
            wop = ctx.enter_context(tc.tile_pool(name="wop", bufs=3))
            wo_pre = {}

            def load_wo(ec, cc):
                woc = wop.tile([128, 512], BF16, tag="wo")
                nc.sync.dma_start(
                    woc, wo[cc * 128:(cc + 1) * 128, ec * 512:(ec + 1) * 512])
                return woc

            for _cc in range(2):
                wo_pre[(0, _cc)] = load_wo(0, _cc)

            # ---------------- Phase A: attention ----------------
            with tc.tile_pool(name="slabp", bufs=3) as slabp, \
                 tc.tile_pool(name="stO", bufs=2) as stO, \
                 tc.tile_pool(name="bcsp", bufs=2) as bcsp, \
                 tc.tile_pool(name="psOB", bufs=3, space="PSUM") as psOB, \
                 tc.tile_pool(name="psB", bufs=1, space="PSUM") as psB, \
                 tc.tile_pool(name="psS", bufs=2, space="PSUM") as psS:

                if DEBUG:
                    nc.sync.dma_start(dqT[:, :], qT[:, :].bitcast(F32))
                    nc.sync.dma_start(dkT[:, :], kT[:, :].bitcast(F32))
                    nc.sync.dma_start(dvsl[:, :], vsl[:, :])

                def score_chunk(h, kv, kc, slab):
                    wlo, whi, woff, wlen = chunk_window(kc)
                    stile = psS.tile([128, SLOT], F32, tag="sc")
                    kstat = kT[0:67, kv * KR + kc * 128:kv * KR + (kc + 1) * 128]
                    w0 = min(wlen, 512)
                    ltri = kc >= 4   # left tri: future keys
                    rtri = kc <= 7   # right tri: beyond-window past keys
                    rtc = wlen - 128  # right tri = last 128 written cols
                    nc.tensor.matmul(
                        stile[:, 0:w0], lhsT=kstat,
                        rhs=qT[0:67, h * RT + wlo:h * RT + wlo + w0],
                        start=True,
                        stop=not (ltri or (rtri and rtc < 512)),
                        skip_group_check=True)
                    if wlen > 512:
                        nc.tensor.matmul(
                            stile[:, 512:wlen], lhsT=kstat,
                            rhs=qT[0:67, h * RT + wlo + 512:h * RT + whi],
                            start=True, stop=not rtri,
                            skip_group_check=True)
                    # Window masks = -1e9 accumulated via PE (lhsT = tri^T,
                    # rhs = I): exp turns them into exact zeros. Left tri =
                    # future keys (positive ALiBi bias would overflow exp);
                    # right tri = beyond-window past keys.
                    if ltri:
                        nc.tensor.matmul(
                            stile[:, 0:128],
                            lhsT=wnegT_sb[:, 0:128], rhs=id_sb,
                            start=False, stop=True,
                            skip_group_check=True)
                    if rtri:
                        nc.tensor.matmul(
                            stile[:, rtc:rtc + 128],
                            lhsT=wnegT_sb[:, 128:256], rhs=id_sb,
                            start=False, stop=True,
                            skip_group_check=True)
                    # No max-subtraction: valid scores are N(0, ~6.5);
                    # exp overflow needs ~13 sigma; bf16 range is ample.
                    nc.scalar.activation(
                        slab[:, kc * SLOT + woff:kc * SLOT + woff + wlen],
                        stile[:, 0:wlen], Exp, bias=0.0)

                def pv(h, kv, half, slab):
                    oT = psOB.tile([VB, 512], F32, tag="ps_ob")
                    # s=3 first: full 512-wide, zeroes the bank
                    order = [3, 0, 1, 2, 4, 5, 6, 7]
                    for i, s in enumerate(order):
                        kc = 4 * half + s
                        c0 = max(0, 128 * s - 512)
                        c1 = min(512, 128 * s + 128)
                        rw0 = max(512 - 128 * s, 0)
                        nc.tensor.matmul(
                            oT[:, c0:c1],
                            lhsT=vsl[:, kc * HKV * VB + kv * VB:
                                     kc * HKV * VB + (kv + 1) * VB],
                            rhs=slab[:, kc * SLOT + rw0:kc * SLOT + rw0 + (c1 - c0)],
                            start=(i == 0), stop=(i == 7),
                            skip_group_check=True)
                    return oT

                _rec_i = [0]

                def norm(h, half, oT, stg_t):
                    cc = h // 2
                    recs = recsT[_rec_i[0] % 2]
                    _rec_i[0] += 1
                    with nc.allow_low_precision(reason="recip rounded to f32r"):
                        nc.vector.reciprocal(recs[64:65, :], oT[64:65, :])
                    bc = psB.tile([64, 512], F32, tag="bc")
                    nc.tensor.matmul(bc, lhsT=bwt[64:66, 0:64],
                                     rhs=recs[64:66, :],
                                     start=True, stop=True)
                    # TT can take at most one PSUM operand: stage bc
                    bcs = bcsp.tile([64, 512], F32, tag="bcs")
                    nc.vector.tensor_copy(bcs, bc)
                    if DEBUG and h == 0:
                        nc.sync.dma_start(
                            drecs[half:half + 1, :], recs[64:65, :].bitcast(F32))
                    cb = cc * RT + half * 512
                    if h % 2 == 0:
                        nc.vector.tensor_mul(
                            attnT[0:64, cb:cb + 512], oT[0:64, :], bcs)
                    else:
                        nc.vector.tensor_mul(
                            stg_t[:, half * 512:(half + 1) * 512],
                            oT[0:64, :], bcs)

                # head pairs (even, odd) interleaved: two independent chains
                # keep every engine busy through the other's sem latencies
                for kv in range(HKV):
                    for gp in range(2):
                        hA = kv * G + 2 * gp
                        hB = hA + 1
                        cc = hA // 2
                        slabA = slabp.tile([128, NKC * SLOT], BF16, tag="slab")
                        slabB = slabp.tile([128, NKC * SLOT], BF16, tag="slab")
                        stg_t = stO.tile([64, 2 * 512], BF16, tag="so")
                        for kc in range(NKC):
                            score_chunk(hA, kv, kc, slabA)
                            score_chunk(hB, kv, kc, slabB)
                        oTA0 = pv(hA, kv, 0, slabA)
                        oTB0 = pv(hB, kv, 0, slabB)
                        norm(hA, 0, oTA0, None)
                        norm(hB, 0, oTB0, stg_t)
                        oTA1 = pv(hA, kv, 1, slabA)
                        oTB1 = pv(hB, kv, 1, slabB)
                        norm(hA, 1, oTA1, None)
                        norm(hB, 1, oTB1, stg_t)
                        nc.gpsimd.dma_start(
                            attnT[64:128, cc * RT:(cc + 1) * RT], stg_t)
                        if DEBUG and hA == 0:
                            nc.sync.dma_start(dslab[:, :], slabA)

                if DEBUG:
                    nc.sync.dma_start(dattnT[:, :], attnT[:, :])

            # ---------------- Phase O: output projection ----------------
            with tc.tile_pool(name="obp", bufs=2) as obp, \
                 tc.tile_pool(name="psF", bufs=1, space="PSUM") as psFp:
                for ec in range(2):
                    psF = psFp.tile([128, 8 * 512], F32, tag="fps")
                    for cc in range(8):
                        woc = wo_pre.pop((ec, cc), None)
                        if woc is None:
                            woc = load_wo(ec, cc)
                        for tk in range(8):
                            nc.tensor.matmul(
                                psF[:, tk * 512:(tk + 1) * 512],
                                lhsT=attnT[:, cc * RT + tk * 128:cc * RT + (tk + 1) * 128],
                                rhs=woc,
                                start=(cc == 0), stop=(cc == 7),
                                skip_group_check=True)
                    ob = obp.tile([128, 8 * 512], F32, tag="ob")
                    for tk in range(8):
                        nc.any.tensor_copy(ob[:, tk * 512:(tk + 1) * 512],
                                           psF[:, tk * 512:(tk + 1) * 512])
                        if tk % 2 == 1:  # store per 2-tk chunk for early overlap
                            nc.scalar.dma_start(
                                _dram_ap(out[:, :], (tk - 1) * 128 * C + ec * 512,
                                         [[C, 128], [128 * C, 2], [1, 512]]),
                                _ap(ob[:, 0:1], (tk - 1) * 512,
                                    [[512, 2], [1, 512]]))

    nc.compile()
    return nc


_NC = None


def _host_inputs(x, wqkv, wo):
    slopes = alibi_slopes(H)  # head h = kv*G + g matches slopes.reshape(HKV, G)

    wqkv_s = np.array(wqkv, dtype=np.float32, copy=True)
    wqkv_s[:, :C] *= SCALE  # exact power-of-two fold of the score scale into wq

    wo_bf = np.asarray(wo, dtype=np.float32).astype(ml_dtypes.bfloat16)

    j = np.arange(RT, dtype=np.float32)
    qaug = np.empty((H, 3, RT), dtype=np.float32)
    for h in range(H):
        qaug[h, 0] = -slopes[h] * (j + 512.0)
        qaug[h, 1] = slopes[h]
        qaug[h, 2] = 1.0

    i = np.arange(KR, dtype=np.float32)
    kaug_base = np.empty((3, KR), dtype=np.float32)
    kaug_base[0] = 1.0
    kaug_base[1] = i
    kaug_base[2] = 0.0

    # 0/1 multiplicative triangle masks (bf16):
    # left strip: invalid (0) where col < part; right strip: invalid where col >= part
    r = np.arange(128)[:, None]
    l = np.arange(128)[None, :]
    wneg_l = np.where(l < r, np.float32(NEG), np.float32(0.0)).astype(np.float32)
    wneg_r = np.where(l >= r, np.float32(NEG), np.float32(0.0)).astype(np.float32)
    wnegT = np.ascontiguousarray(
        np.concatenate([wneg_l.T, wneg_r.T], axis=1)).astype(np.float32)
    ident = np.eye(128, dtype=np.float32)

    in_maps = []
    for core in range(NCORES):
        b, qq = core // 4, core % 4
        t0 = qq * RT
        xsl = np.zeros((KR, C), dtype=np.float32)
        lo = t0 - W
        if lo < 0:
            xsl[-lo:, :] = x[b, 0:t0 + RT, :]
        else:
            xsl[:, :] = x[b, lo:t0 + RT, :]
        xsl = np.ascontiguousarray(xsl.T)
        kaug = kaug_base.copy()
        if lo < 0:
            kaug[2, :W] = NEG  # left-edge penalty kills padded keys
        kaug4 = np.ascontiguousarray(np.tile(kaug, (1, HKV)))
        cns = np.zeros((3, 512), dtype=np.float32)
        cns[0] = 1.0
        in_maps.append(dict(xs=xsl, wqkv=wqkv_s, wo=wo_bf,
                            qaug=qaug, kaug=kaug4, wnegT=wnegT, ident=ident,
                            cns=cns))
    return in_maps


def kernel(x, wqkv, wo):
    global _NC
    if _NC is None:
        _NC = build_nc()
    in_maps = _host_inputs(np.asarray(x), np.asarray(wqkv), np.asarray(wo))
    res = run_bass_kernel_spmd(_NC, in_maps, list(range(NCORES)))
    full = np.empty((B, T, C), dtype=np.float32)
    for core in range(NCORES):
        b, qq = core // 4, core % 4
        full[b, qq * RT:(qq + 1) * RT, :] = res.results[core]["out"]
    return full
